# revision 1
# baseline (speedup 1.0000x reference)
"""Causal self-attention with rotary embeddings on 8 Trainium2 NeuronCores.

Tensor-parallel over heads: 16 heads / 8 cores = 2 heads per core.
Each core computes qkv for its 2 heads, rotary, causal attention, and a
partial output projection (its 128 rows of w_proj); the host sums the 8
partial outputs.

Device-side layout (per core, heads A/B local):
  - Everything "transposed": Q^T/K^T stored [d(128=A:0-63,B:64-127), t(4096)].
  - Scores computed as S^T = K_blk @ Q^T  -> [k(128), q] so softmax's k-sum
    can be folded into the P@V matmul via a ones-augmented V (extra lhsT
    column of ones produces the denominator row). No max-subtraction is
    needed (scores are O(6) for this distribution; fp32 exp is safe).
  - Rotary applied in the transposed layout via a pair-swap permutation
    matmul: rot(q) = cos_exp * q + sin_sgn * (Pswap @ q).
  - V transposed to t-major [k, d] tiles with the PE transpose path.

All matmul inputs fp16 (1 cyc/row on PE); accumulation fp32 in PSUM.
"""

import numpy as np

B, T, C, H = 2, 2048, 1024, 16
HD = C // H            # 64
N_CORES = 8
HPC = H // N_CORES     # 2 heads per core
BT = B * T             # 4096
TC = 512               # t-chunk for phase 1 (qkv/rotary)
NTC = BT // TC         # 8
KB = 128               # k-block size
NKB = T // KB          # 16 k-blocks per batch
QC = 512               # q-chunk for PV accumulation
NQC = T // QC          # 4

_CACHE = {}


def _build_bass(debug=False):
    import concourse.bacc as bacc
    import concourse.mybir as mybir
    import concourse.tile as tile
    from concourse.masks import make_identity, make_upper_triangular

    f16 = mybir.dt.float16
    f32 = mybir.dt.float32

    nc = bacc.Bacc()

    if debug:
        dbg_qrot = nc.dram_tensor("dbg_qrot", [128, BT], f16,
                                  kind="ExternalOutput")
        dbg_krot = nc.dram_tensor("dbg_krot", [128, BT], f16,
                                  kind="ExternalOutput")
        dbg_vaug = nc.dram_tensor("dbg_vaug", [128, 2 * NKB * 130], f16,
                                  kind="ExternalOutput")
        dbg_yn = nc.dram_tensor("dbg_yn", [128, B * T], f16,
                                kind="ExternalOutput")
        dbg_p = nc.dram_tensor("dbg_p", [128, T], f16, kind="ExternalOutput")
        dbg_den = nc.dram_tensor("dbg_den", [128, QC], f32,
                                 kind="ExternalOutput")

    xT = nc.dram_tensor("xT", [C, BT], f16, kind="ExternalInput")
    wqkv = nc.dram_tensor("wqkv", [C, 3 * HPC * HD], f16, kind="ExternalInput")
    wp = nc.dram_tensor("wp", [HPC * HD, C], f16, kind="ExternalInput")
    cos_e = nc.dram_tensor("cos_e", [128, BT], f16, kind="ExternalInput")
    sin_e = nc.dram_tensor("sin_e", [128, BT], f16, kind="ExternalInput")
    pswap = nc.dram_tensor("pswap", [128, 128], f16, kind="ExternalInput")
    y = nc.dram_tensor("y", [BT, C], f16, kind="ExternalOutput")

    CCH = C // 128  # 8 contraction chunks

    with tile.TileContext(nc) as tc:
        with (
            tc.tile_pool(name="const", bufs=1) as const,
            tc.tile_pool(name="persist", bufs=1) as persist,
            tc.tile_pool(name="ptiles", bufs=18) as ptiles,
            tc.tile_pool(name="stream", bufs=2) as stream,
            tc.tile_pool(name="psum", bufs=1, space="PSUM") as psum,
        ):
            # ---- constants ----
            wqkv_sb = const.tile([128, CCH, 384], f16)
            wqkv_r = wqkv.rearrange("(cc p) j -> p cc j", p=128)
            for cc in range(CCH):
                nc.sync.dma_start(out=wqkv_sb[:, cc, :], in_=wqkv_r[:, cc, :])
            wp_sb = const.tile([128, C], f16)
            nc.sync.dma_start(out=wp_sb, in_=wp[:, :])
            pswap_sb = const.tile([128, 128], f16)
            nc.sync.dma_start(out=pswap_sb, in_=pswap[:, :])
            ident = const.tile([128, 128], f16)
            make_identity(nc, ident)
            # mask[k, q] = 1 where q >= k (keep), 0 where q < k
            mask_ut = const.tile([128, 128], f16)
            make_upper_triangular(nc, mask_ut, val=1.0, diag=True)

            # ---- persistent tensors ----
            QrotT = persist.tile([128, BT], f16)
            KrotT = persist.tile([128, BT], f16)
            # V in t-major, per k-block: [V_A(64) | ones | V_B(64) | ones]
            Vaug = persist.tile([128, 2 * NKB, 130], f16)
            Yn = persist.tile([128, B, T], f16)
            ones_cols = Vaug.rearrange("p J (h x) -> p J h x", x=65)[:, :, :, 64]
            nc.gpsimd.memset(ones_cols, 1.0)

            xT_r = xT.rearrange("(cc p) t -> p cc t", p=128)

            # ================= phase 1: qkv + rotary + V transpose ========
            for i in range(NTC):
                ts = slice(i * TC, (i + 1) * TC)
                x_sb = stream.tile([128, CCH, TC], f16, tag="x")
                for cc in range(CCH):
                    nc.sync.dma_start(out=x_sb[:, cc, :], in_=xT_r[:, cc, ts])
                cos_sb = stream.tile([128, TC], f16, tag="cos")
                sin_sb = stream.tile([128, TC], f16, tag="sin")
                nc.sync.dma_start(out=cos_sb, in_=cos_e[:, ts])
                nc.sync.dma_start(out=sin_sb, in_=sin_e[:, ts])

                for g in range(3):  # Q, K, V groups
                    acc = psum.tile([128, TC], f32, tag="mm512", bufs=2)
                    for cc in range(CCH):
                        nc.tensor.matmul(
                            acc, wqkv_sb[:, cc, g * 128:(g + 1) * 128],
                            x_sb[:, cc, :],
                            start=(cc == 0), stop=(cc == CCH - 1))
                    if g < 2:  # Q or K: rotary
                        dst = QrotT if g == 0 else KrotT
                        graw = stream.tile([128, TC], f16, tag="graw")
                        nc.vector.tensor_copy(graw, acc)
                        swp = psum.tile([128, TC], f32, tag="mm512", bufs=2)
                        nc.tensor.matmul(swp, pswap_sb, graw,
                                         start=True, stop=True)
                        t1 = stream.tile([128, TC], f16, tag="t1")
                        nc.vector.tensor_mul(t1, graw, cos_sb)
                        t2 = stream.tile([128, TC], f16, tag="t2")
                        nc.vector.tensor_mul(t2, swp, sin_sb)
                        nc.vector.tensor_add(dst[:, ts], t1, t2)
                    else:  # V: transpose to t-major
                        vtmp = stream.tile([128, TC], f16, tag="vtmp")
                        nc.vector.tensor_copy(vtmp, acc)
                        for q in range(TC // 128):
                            J = i * (TC // 128) + q
                            vt = psum.tile([128, 128], f16, tag="mm512",
                                           bufs=2, name="vt")
                            nc.tensor.transpose(
                                vt, vtmp[:, q * 128:(q + 1) * 128], ident)
                            vdst = Vaug.rearrange(
                                "p J (h x) -> p J h x", x=65)[:, J, :, 0:64]
                            vsrc = vt.rearrange("p (h x) -> p h x", h=2)
                            nc.vector.tensor_copy(vdst, vsrc)

            # ================= phase 2: attention =========================
            for b in range(B):
                qoff = b * T
                for h in range(HPC):
                    hs = slice(h * 64, (h + 1) * 64)
                    p_tiles = []
                    for j in range(NKB):
                        L = T - j * KB
                        pt = ptiles.tile([128, T], f16, tag="pt", name="pt")
                        for k0 in range(0, L, 1024):
                            kl = min(1024, L - k0)
                            st = psum.tile([128, 1024], f32, tag="st",
                                           bufs=2, name="st")
                            for s0 in range(0, kl, 512):
                                sl = min(512, kl - s0)
                                nc.tensor.matmul(
                                    st[:, s0:s0 + sl],
                                    KrotT[hs,
                                          qoff + j * KB: qoff + j * KB + 128],
                                    QrotT[hs, qoff + j * KB + k0 + s0:
                                          qoff + j * KB + k0 + s0 + sl],
                                    start=True, stop=True)
                            nc.scalar.activation(
                                pt[:, j * KB + k0: j * KB + k0 + kl],
                                st[:, 0:kl],
                                mybir.ActivationFunctionType.Exp)
                        # causal mask inside the diagonal block
                        nc.vector.tensor_mul(
                            pt[:, j * KB: j * KB + 128],
                            pt[:, j * KB: j * KB + 128], mask_ut)
                        # zero-pad back to the enclosing q-chunk boundary
                        pad = (j % 4) * KB
                        if pad:
                            nc.gpsimd.memset(
                                pt[:, (j - j % 4) * KB: j * KB], 0.0)
                        if debug and b == 0 and h == 0 and j == 0:
                            nc.sync.dma_start(out=dbg_p[:, :], in_=pt)
                        p_tiles.append(pt)

                    for c in range(NQC):
                        yps = psum.tile([128, QC], f32, tag="y", bufs=2,
                                        name="yps")
                        jmax = 4 * c + 3
                        for j in range(jmax + 1):
                            J = b * NKB + j
                            nc.tensor.matmul(
                                yps[0:65, :],
                                Vaug[:, J, h * 65:(h + 1) * 65],
                                p_tiles[j][:, c * QC:(c + 1) * QC],
                                start=(j == 0), stop=(j == jmax))
                        # normalize: rows 0-63 divided by the ones-row (64)
                        recip = stream.tile([128, QC], f32, tag="recip")
                        # hw partition_broadcast reads the tensor's partition
                        # 0 regardless of AP base, so put 1/den on row 0
                        # (DVE cross-partition in@64 -> out@0 works).
                        # stage den into SBUF first: the custom-DVE
                        # reciprocal misreads PSUM/cross-partition inputs
                        dsb = stream.tile([128, QC], f32, tag="dsb")
                        nc.vector.tensor_copy(dsb[0:1, :], yps[64:65, :])
                        nc.vector.reciprocal_approx_fast(
                            out=recip[0:1, :], in_=dsb[0:1, :])
                        bc = stream.tile([128, QC], f32, tag="bc")
                        nc.gpsimd.partition_broadcast(
                            bc[0:64, :], recip[0:1, :])
                        if debug and b == 0 and h == 0 and c == 0:
                            nc.sync.dma_start(out=dbg_den[:, :], in_=bc)
                        if h == 0:
                            nc.vector.tensor_tensor(
                                out=Yn[0:64, b, c * QC:(c + 1) * QC],
                                in0=yps[0:64, :], in1=bc[0:64, :],
                                op=mybir.AluOpType.mult)
                        else:
                            ytmp = stream.tile([128, QC], f16, tag="ytmp")
                            nc.vector.tensor_tensor(
                                out=ytmp[0:64, :],
                                in0=yps[0:64, :], in1=bc[0:64, :],
                                op=mybir.AluOpType.mult)
                            # cross-partition move 0-63 -> 64-127 via DMA
                            nc.sync.dma_start(
                                out=Yn[64:128, b, c * QC:(c + 1) * QC],
                                in_=ytmp[0:64, :])

                # ---- projection for batch b ----
                for tt in range(T // 128):
                    for half in range(2):
                        pout = psum.tile([128, 512], f32, tag="mm512",
                                         bufs=2, name="pout")
                        nc.tensor.matmul(
                            pout, Yn[:, b, tt * 128:(tt + 1) * 128],
                            wp_sb[:, half * 512:(half + 1) * 512],
                            start=True, stop=True)
                        yout = stream.tile([128, 512], f16, tag="yo")
                        nc.vector.tensor_copy(yout, pout)
                        nc.sync.dma_start(
                            out=y[qoff + tt * 128: qoff + (tt + 1) * 128,
                                  half * 512:(half + 1) * 512],
                            in_=yout)

            if debug:
                nc.sync.dma_start(out=dbg_qrot[:, :], in_=QrotT)
                nc.sync.dma_start(out=dbg_krot[:, :], in_=KrotT)
                nc.sync.dma_start(
                    out=dbg_vaug[:, :],
                    in_=Vaug.rearrange("p J x -> p (J x)"))
                nc.sync.dma_start(out=dbg_yn[:, :],
                                  in_=Yn.rearrange("p b t -> p (b t)"))

    nc.finalize()
    return nc


def _host_prep(x, cos, sin, w_attn, b_attn, w_proj):
    """Shared + per-core input arrays (all fp16 except noted)."""
    x2 = np.asarray(x, dtype=np.float32).reshape(BT, C)
    xT16 = np.ascontiguousarray(x2.T).astype(np.float16)

    cos = np.asarray(cos, dtype=np.float32)
    sin = np.asarray(sin, dtype=np.float32)
    d = np.arange(128) % 64
    freq_i = d // 2
    sign = np.where(d % 2 == 0, -1.0, 1.0).astype(np.float32)
    cos_exp = np.tile(cos[:, freq_i].T, (1, B)).astype(np.float16)  # [128, BT]
    sin_exp = (sign[:, None] * np.tile(sin[:, freq_i].T, (1, B))).astype(
        np.float16)

    pswap = np.zeros((128, 128), dtype=np.float16)
    idx = np.arange(128)
    pswap[idx ^ 1, idx] = 1.0

    w_attn = np.asarray(w_attn, dtype=np.float32)
    w_proj = np.asarray(w_proj, dtype=np.float32)
    scale = 1.0 / np.sqrt(HD)

    per_core = []
    for m in range(N_CORES):
        cols = []
        for g in range(3):          # q, k, v blocks of w_attn
            for hh in range(HPC):
                hglob = m * HPC + hh
                blk = w_attn[:, g * C + hglob * HD:(g * C + (hglob + 1) * HD)]
                if g == 0:
                    blk = blk * scale
                cols.append(blk)
        w_stack = np.concatenate(cols, axis=1).astype(np.float16)
        wp_m = w_proj[m * HPC * HD:(m + 1) * HPC * HD, :].astype(np.float16)
        per_core.append((w_stack, wp_m))
    return xT16, cos_exp, sin_exp, pswap, per_core


def kernel(x, cos, sin, w_attn, b_attn, w_proj, b_proj):
    from concourse.bass_utils import run_bass_kernel_spmd

    b_attn = np.asarray(b_attn, dtype=np.float32)
    assert not np.any(b_attn), "nonzero b_attn not supported by this kernel"

    xT16, cos_exp, sin_exp, pswap, per_core = _host_prep(
        x, cos, sin, w_attn, b_attn, w_proj)

    if "nc" not in _CACHE:
        _CACHE["nc"] = _build_bass()
    nc = _CACHE["nc"]

    in_maps = []
    for m in range(N_CORES):
        w_stack, wp_m = per_core[m]
        in_maps.append({
            "xT": xT16, "wqkv": w_stack, "wp": wp_m,
            "cos_e": cos_exp, "sin_e": sin_exp, "pswap": pswap,
        })

    res = run_bass_kernel_spmd(nc, in_maps, core_ids=list(range(N_CORES)))
    _CACHE["last_result"] = res

    y = np.zeros((BT, C), dtype=np.float64)
    for m in range(N_CORES):
        y += res.results[m]["y"].astype(np.float64)
    y = y + np.asarray(b_proj, dtype=np.float64)[None, :]
    return y.reshape(B, T, C).astype(np.float32)



# revision 3
# speedup vs baseline: 1.0661x; 1.0661x over previous
"""Causal self-attention with rotary embeddings on 8 Trainium2 NeuronCores.

Hybrid batch+head tensor parallel: core m handles batch m//4 and heads
[4*(m%4), 4*(m%4)+4).  Each core reads only its batch's x (4 MB), computes
qkv for its 4 heads, rotary, causal attention, and a partial output
projection with its 256 rows of w_proj; the host sums the 4 partial
outputs per batch.

Per-core device layout (heads grouped in 2 pairs):
  - Q^T/K^T per pair: [128 rows = head_a(64) | head_b(64), t].  Within a
    head the 64 dims are permuted to [evens(32), odds(32)] (host permutes
    the w_attn columns), making rotary 3 fast fp16 DVE tensor ops plus a
    32-row block swap done by SBUF->SBUF DMA.  Scores are invariant to a
    shared d-permutation of Q and K.
  - Scores S^T = K_blk @ Q^T -> [k(128), q]; exp on the scalar engine; a
    ones-augmented V makes row 64 of the P@V accumulation the softmax
    denominator.  No max-subtraction (scores are O(6); fp32 exp is safe).
  - V is computed directly in t-major orientation (x block stationary,
    w_v moving) - no PE transposes.
  - Attention runs chunk-major (512 queries at a time); the output
    projection for a chunk runs right after its normalize, so the y DMA
    streams through phase 2 instead of forming a tail.

All matmul inputs fp16 (1 cyc/row on PE); accumulation fp32 in PSUM.
"""

import numpy as np

B, T, C, H = 2, 2048, 1024, 16
HD = C // H            # 64
N_CORES = 8
CPB = 4                # cores per batch
HPC = 4                # heads per core (2 pairs)
TC = 512               # t-chunk for phase 1
NTC = T // TC          # 4
KB = 128               # k-block
NKB = T // KB          # 16
QC = 512               # q-chunk for attention/projection
NQC = T // QC          # 4

_CACHE = {}


def _build_bass():
    import concourse.bacc as bacc
    import concourse.mybir as mybir
    import concourse.tile as tile
    from concourse.masks import make_upper_triangular

    f16 = mybir.dt.float16
    f32 = mybir.dt.float32
    Exp = mybir.ActivationFunctionType.Exp
    Copy = mybir.ActivationFunctionType.Copy
    mult = mybir.AluOpType.mult

    nc = bacc.Bacc()

    xT = nc.dram_tensor("xT", [C, T], f16, kind="ExternalInput")
    wqkv = nc.dram_tensor("wqkv", [C, 768], f16, kind="ExternalInput")
    wp = nc.dram_tensor("wp", [2 * 128, C], f16, kind="ExternalInput")
    trig1 = nc.dram_tensor("trig1", [128, T], f16, kind="ExternalInput")
    trig2 = nc.dram_tensor("trig2", [128, T], f16, kind="ExternalInput")
    y = nc.dram_tensor("y", [T, C], f16, kind="ExternalOutput")

    CCH = C // 128  # 8 contraction chunks

    with tile.TileContext(nc) as tc:
        with (
            tc.tile_pool(name="const", bufs=1) as const,
            tc.tile_pool(name="persist", bufs=1) as persist,
            tc.tile_pool(name="stream", bufs=2) as stream,
            tc.tile_pool(name="ptp", bufs=18) as ptp,
            tc.tile_pool(name="psum", bufs=1, space="PSUM") as psum,
        ):
            # ---- constants ----
            wqkv_sb = const.tile([128, CCH, 768], f16)
            wqkv_r = wqkv.rearrange("(cc p) j -> p cc j", p=128)
            for cc in range(CCH):
                nc.sync.dma_start(out=wqkv_sb[:, cc, :], in_=wqkv_r[:, cc, :])
            trig1_sb = const.tile([128, T], f16)
            nc.sync.dma_start(out=trig1_sb, in_=trig1[:, :])
            trig2_sb = const.tile([128, T], f16)
            nc.sync.dma_start(out=trig2_sb, in_=trig2[:, :])
            wp_sb = const.tile([128, 2, C], f16)
            wp_r = wp.rearrange("(p2 p) c -> p p2 c", p=128)
            nc.sync.dma_start(out=wp_sb, in_=wp_r)
            # mask[k, q] = 1 where q >= k (keep), 0 where q < k
            mask_ut = const.tile([128, 128], f16)
            make_upper_triangular(nc, mask_ut, val=1.0, diag=True)

            # ---- persistent tensors ----
            QrotT = persist.tile([128, 2, T], f16)
            KrotT = persist.tile([128, 2, T], f16)
            # V in t-major per (pair, k-block): [V_a(64) | ones | V_b(64) | ones]
            Vaug = persist.tile([128, 2, NKB, 130], f16)
            ones_cols = Vaug.rearrange(
                "pp q J (h x) -> pp q J h x", x=65)[:, :, :, :, 64]
            nc.gpsimd.memset(ones_cols, 1.0)
            Yn = persist.tile([128, 2, T], f16)

            xT_r = xT.rearrange("(cc p) t -> p cc t", p=128)

            # ================= phase 1: qkv + rotary ======================
            for i in range(NTC):
                ts = slice(i * TC, (i + 1) * TC)
                x_sb = stream.tile([128, CCH, TC], f16, tag="x")
                for cc in range(CCH):
                    nc.sync.dma_start(out=x_sb[:, cc, :], in_=xT_r[:, cc, ts])

                for g in range(4):      # Qp0 Qp1 Kp0 Kp1
                    dst = QrotT if g < 2 else KrotT
                    p = g % 2
                    acc = psum.tile([128, TC], f32, tag="acc", bufs=2,
                                    name="acc")
                    for cc in range(CCH):
                        nc.tensor.matmul(
                            acc, wqkv_sb[:, cc, g * 128:(g + 1) * 128],
                            x_sb[:, cc, :],
                            start=(cc == 0), stop=(cc == CCH - 1))
                    g16 = stream.tile([128, TC], f16, tag="g16")
                    nc.scalar.activation(g16, acc, Copy)
                    # 32-row block swap (evens <-> odds per head) via DMA
                    gsw = stream.tile([128, TC], f16, tag="gsw")
                    for blk in range(4):
                        src = blk ^ 1
                        nc.sync.dma_start(
                            out=gsw[blk * 32:(blk + 1) * 32, :],
                            in_=g16[src * 32:(src + 1) * 32, :])
                    m1 = stream.tile([128, TC], f16, tag="m1")
                    nc.vector.tensor_mul(m1, g16, trig1_sb[:, ts])
                    m2 = stream.tile([128, TC], f16, tag="m2")
                    nc.vector.tensor_mul(m2, gsw, trig2_sb[:, ts])
                    nc.vector.tensor_add(dst[:, p, ts], m1, m2)

                # V direct to t-major: x block stationary, w_v moving
                for tb in range(TC // 128):
                    J = i * 4 + tb
                    vacc = psum.tile([128, TC], f32, tag="acc", bufs=2,
                                     name="vacc")
                    for cc in range(CCH):
                        nc.tensor.matmul(
                            vacc[:, 0:256],
                            x_sb[:, cc, tb * 128:(tb + 1) * 128],
                            wqkv_sb[:, cc, 512:768],
                            start=(cc == 0), stop=(cc == CCH - 1))
                    for p in range(2):
                        vdst = Vaug.rearrange(
                            "pp q J (h x) -> pp q J h x",
                            x=65)[:, p, J, :, 0:64]
                        vsrc = vacc[:, p * 128:(p + 1) * 128].rearrange(
                            "pp (h x) -> pp h x", h=2)
                        nc.scalar.activation(vdst, vsrc, Copy)

            # ================= phase 2: attention, chunk-major ============
            for c in range(NQC):
                cs = slice(c * QC, (c + 1) * QC)
                for u in range(HPC):
                    p, hh = divmod(u, 2)
                    hs = slice(hh * 64, hh * 64 + 64)
                    jmax = 4 * c + 3

                    # scores + exp (+ diag mask) for all k-blocks of chunk
                    pts = []
                    for j in range(jmax + 1):
                        prefix = max(0, (j - 4 * c) * 128)
                        st = psum.tile([128, QC], f32, tag="st", bufs=2,
                                       name="st")
                        nc.tensor.matmul(
                            st[:, prefix:],
                            KrotT[hs, p, j * KB:(j + 1) * KB],
                            QrotT[hs, p, c * QC + prefix:(c + 1) * QC],
                            start=True, stop=True)
                        pt = ptp.tile([128, QC], f16, tag="pt", name="pt")
                        if prefix:
                            nc.gpsimd.memset(pt[:, 0:prefix], 0.0)
                        nc.scalar.activation(pt[:, prefix:], st[:, prefix:],
                                             Exp)
                        if j >= 4 * c:
                            nc.vector.tensor_mul(
                                pt[:, prefix:prefix + 128],
                                pt[:, prefix:prefix + 128], mask_ut)
                        pts.append(pt)

                    # P @ V_aug accumulation; order so every column's first
                    # writer has start=True and last writer has stop=True:
                    # j=0 (full) first, partial diagonal blocks in the
                    # middle, j=4c (full) last.
                    yps = psum.tile([128, QC], f32, tag="yps", bufs=2,
                                    name="yps")
                    if c == 0:
                        order = [(j, j == 0, j == 3, 0) for j in range(4)]
                    else:
                        order = [(j, j == 0, False, 0) for j in range(4 * c)]
                        order += [(j, False, False, (j - 4 * c) * 128)
                                  for j in range(4 * c + 1, 4 * c + 4)]
                        order += [(4 * c, False, True, 0)]
                    for j, sa, so, pvlo in order:
                        nc.tensor.matmul(
                            yps[0:65, pvlo:],
                            Vaug[:, p, j, hh * 65:(hh + 1) * 65],
                            pts[j][:, pvlo:],
                            start=sa, stop=so)

                    # normalize rows 0-63 by the ones-row (64)
                    dsb = stream.tile([128, QC], f32, tag="dsb")
                    nc.vector.tensor_copy(dsb[0:1, :], yps[64:65, :])
                    rcp = stream.tile([128, QC], f32, tag="rcp")
                    nc.vector.reciprocal_approx_fast(out=rcp[0:1, :],
                                                     in_=dsb[0:1, :])
                    bc = stream.tile([128, QC], f32, tag="bc")
                    nc.gpsimd.partition_broadcast(bc[0:64, :], rcp[0:1, :])
                    nc.vector.tensor_tensor(
                        out=Yn[hs, p, cs], in0=yps[0:64, :], in1=bc[0:64, :],
                        op=mult)

                # ---- projection + output DMA for this chunk ----
                for tt in range(4 * c, 4 * c + 4):
                    tsl = slice(tt * 128, (tt + 1) * 128)
                    for half in range(2):
                        hsl = slice(half * 512, (half + 1) * 512)
                        pout = psum.tile([128, 512], f32, tag="pout", bufs=2,
                                         name="pout")
                        nc.tensor.matmul(pout, Yn[:, 0, tsl],
                                         wp_sb[:, 0, hsl],
                                         start=True, stop=False)
                        nc.tensor.matmul(pout, Yn[:, 1, tsl],
                                         wp_sb[:, 1, hsl],
                                         start=False, stop=True)
                        yo = stream.tile([128, 512], f16, tag="yo", bufs=4)
                        if half == 0:
                            nc.vector.tensor_copy(yo, pout)
                        else:
                            nc.scalar.activation(yo, pout, Copy)
                        nc.sync.dma_start(out=y[tsl, hsl], in_=yo)

    nc.finalize()
    return nc


def _host_prep(x, cos, sin, w_attn, b_attn, w_proj):
    """Per-core input maps (all fp16)."""
    x = np.asarray(x, dtype=np.float32)
    xT16 = [np.ascontiguousarray(x[b].T).astype(np.float16) for b in range(B)]

    cos = np.asarray(cos, dtype=np.float32)  # [T, 32]
    sin = np.asarray(sin, dtype=np.float32)
    cosF = cos.T.astype(np.float16)          # [32, T]
    sinF = sin.T.astype(np.float16)
    trig1 = np.concatenate([cosF, cosF, cosF, cosF], axis=0)   # [128, T]
    trig2 = np.concatenate([-sinF, sinF, -sinF, sinF], axis=0)

    w_attn = np.asarray(w_attn, dtype=np.float32)
    w_proj = np.asarray(w_proj, dtype=np.float32)
    scale = 1.0 / np.sqrt(HD)

    # per-head column permutation: [even dims, odd dims]
    perm = np.concatenate([np.arange(0, HD, 2), np.arange(1, HD, 2)])

    in_maps = []
    for m in range(N_CORES):
        hb = (m % CPB) * HPC
        cols = []
        for g in range(2):           # Q, K: permuted dims, Q scaled
            for pp in range(2):
                for hh in range(2):
                    hglob = hb + pp * 2 + hh
                    blk = w_attn[:, g * C + hglob * HD:
                                 g * C + (hglob + 1) * HD][:, perm]
                    if g == 0:
                        blk = blk * scale
                    cols.append(blk)
        for hh in range(HPC):        # V: natural dims
            hglob = hb + hh
            cols.append(w_attn[:, 2 * C + hglob * HD:
                               2 * C + (hglob + 1) * HD])
        w_stack = np.concatenate(cols, axis=1).astype(np.float16)
        wp_m = w_proj[hb * HD:(hb + HPC) * HD, :].astype(np.float16)
        in_maps.append({"xT": xT16[m // CPB], "wqkv": w_stack, "wp": wp_m,
                        "trig1": trig1, "trig2": trig2})
    return in_maps


def kernel(x, cos, sin, w_attn, b_attn, w_proj, b_proj):
    from concourse.bass_utils import run_bass_kernel_spmd

    b_attn = np.asarray(b_attn, dtype=np.float32)
    assert not np.any(b_attn), "nonzero b_attn not supported by this kernel"

    in_maps = _host_prep(x, cos, sin, w_attn, b_attn, w_proj)

    if "nc" not in _CACHE:
        _CACHE["nc"] = _build_bass()
    nc = _CACHE["nc"]

    res = run_bass_kernel_spmd(nc, in_maps, core_ids=list(range(N_CORES)))
    _CACHE["last_result"] = res

    y = np.zeros((B, T, C), dtype=np.float64)
    for m in range(N_CORES):
        y[m // CPB] += res.results[m]["y"].astype(np.float64)
    y += np.asarray(b_proj, dtype=np.float64)[None, None, :]
    return y.astype(np.float32)
